# revision 2
# baseline (speedup 1.0000x reference)
"""Trainium2 Bass kernel for dilated local attention (redesign).

Problem: q,k,v [B=8, d=768, N=6144] fp32; head_dim=32, kernel_size=3.
Per (batch, head, window g) a 3x3 attention over 32-dim head vectors where
window g groups tokens {g, g+2048, g+4096}. Output [B, N, d] with token
n = 3g + (hh//8) and channel 96*(hh%8) + 32*i + cc.

Sharding: batch b -> core b (8 NeuronCores, no communication).

Host casts q,k,v to bf16 (device loads are then cast-free and run on the
SP queue) and upcasts the fp16 output.

Per-core dataflow, per block (gc: 512 windows, cb: 128 channels = 4 heads):
  - SP:  3 load DMAs [128c, 3i, 512g] bf16
  - DVE: tmp[ij] = q_i * k_j  (one op, free 4608, 2x mode)
  - PE:  9 score matmuls, shared per-j weights W_j[128,32] summing each
         32-row head segment, j-outer accumulation -> S[96,512] fp32
         (rows 32i+4j+h)
  - ACT: E = exp(scale*S) -> bf16
  - PE:  D replicated to all rows via wd[96,96] -> fp32 PSUM
  - DVE: Dinv = reciprocal_approx_fast(D)
  - Pool: P = E * Dinv (bf16)
  - PE:  6 broadcast matmuls (is_transpose, one-hot weights) -> bf16 PSUM
         br[c,g] = P[32i+4j+h(c), g]
  - Pool: dv[jj] = v_j - v_1  (j in {0,2})
  - DVE: t4 = br * dv  (one op, free 3072, 2x on bf16 PSUM operand)
  - PE:  36 accumulating matmuls vs I128: t_ps[g, (i,gs,c)] =
         (t4_i0 + t4_i2 + v_1)^T  in fp32 PSUM
  - ACT: copy t_ps -> osb fp16 in output channel order
  - ACT: 1 store DMA per gc ([128g, 4gs, 3t, 768c] fp16, contiguous rows)
"""

import sys

if "/opt/trn_rl_repo" not in sys.path:
    sys.path.insert(0, "/opt/trn_rl_repo")

from contextlib import ExitStack

import numpy as np

import concourse.bacc as bacc
import concourse.tile as tile
from concourse import mybir
from concourse.bass_utils import run_bass_kernel_spmd

B, D, N = 8, 768, 6144
HD, KS = 32, 3
H = D // HD  # 24 heads
G = N // KS  # 2048 windows
NCORES = 8
SCALE = float(HD) ** -0.5

CB = 6   # channel blocks of 128 (4 heads each)
GC = 4   # g-chunks of 512
F = 512  # windows per block
GS = 4   # 128-wide g subchunks per block

F32 = mybir.dt.float32
BF16 = mybir.dt.bfloat16
FP16 = mybir.dt.float16

IJ = [(0, 0), (0, 2), (1, 0), (1, 2), (2, 0), (2, 2)]

_CACHE: dict = {}


def _host_consts():
    # per-j score weights: lhsT [128, 32], out row (4j + p//32) within the
    # 32-row block at partition offset 32i
    wsc = np.zeros((KS, 128, 32), np.float32)
    for j in range(KS):
        for p in range(128):
            wsc[j, p, 4 * j + p // 32] = 1.0
    # D replication: out[32i+4j+h] = sum_j' E[32i+4j'+h]; unused rows get
    # row 32i (finite, harmless)
    wd = np.zeros((96, 96), np.float32)
    for i in range(KS):
        for h in range(4):
            for j in range(KS):
                for jp in range(KS):
                    wd[32 * i + 4 * jp + h, 32 * i + 4 * j + h] = 1.0
    for m in range(96):
        if m % 32 >= 12:
            wd[32 * (m // 32), m] = 1.0
    # broadcast weights: out col c <- P row 32i+4j+(c//32)
    wbr = np.zeros((len(IJ), 96, 128), np.float32)
    for idx, (i, j) in enumerate(IJ):
        for c in range(128):
            wbr[idx, 32 * i + 4 * j + c // 32, c] = 1.0
    ident = np.eye(128, dtype=np.float32)
    return wsc, wd, wbr, ident


def _build_kernel(ctx: ExitStack, tc: tile.TileContext, q, k, v, out, wsc, wd, wbr, ident):
    nc = tc.nc

    consts = ctx.enter_context(tc.tile_pool(name="consts", bufs=1))
    qkv_pool = ctx.enter_context(tc.tile_pool(name="qkv", bufs=6))
    tmp_pool = ctx.enter_context(tc.tile_pool(name="tmp", bufs=3))
    sm_pool = ctx.enter_context(tc.tile_pool(name="sm", bufs=3))
    t4_pool = ctx.enter_context(tc.tile_pool(name="t4", bufs=3))
    out_pool = ctx.enter_context(tc.tile_pool(name="outsb", bufs=2))
    ps_s = ctx.enter_context(tc.tile_pool(name="psS", bufs=1, space="PSUM"))
    ps_d = ctx.enter_context(tc.tile_pool(name="psD", bufs=1, space="PSUM"))
    ps_br = ctx.enter_context(tc.tile_pool(name="psBr", bufs=1, space="PSUM"))
    ps_t = ctx.enter_context(tc.tile_pool(name="psT", bufs=1, space="PSUM"))

    wsc_sb = consts.tile([128, KS, 32], BF16)
    nc.sync.dma_start(out=wsc_sb, in_=wsc.rearrange("n p f -> p n f"))
    wd_sb = consts.tile([96, 96], BF16)
    nc.sync.dma_start(out=wd_sb, in_=wd)
    wbr_sb = consts.tile([96, len(IJ), 128], BF16)
    nc.sync.dma_start(out=wbr_sb, in_=wbr.rearrange("n p f -> p n f"))
    id_sb = consts.tile([128, 128], BF16)
    nc.sync.dma_start(out=id_sb, in_=ident)

    # out viewed [gc, p(g-sub), gs, t, d]; token n = 3*(512gc+128gs+p) + t
    out_r = out[:, :].rearrange(
        "(gc gs p t) d -> gc p gs t d", gc=GC, gs=GS, p=128, t=KS
    )

    def loads(blk):
        gc, cb = blk
        g0, c0 = gc * F, cb * 128
        qsb = qkv_pool.tile([128, KS, F], BF16, tag="q", name=f"q_{gc}_{cb}")
        ksb = qkv_pool.tile([128, KS, F], BF16, tag="k", name=f"k_{gc}_{cb}")
        vsb = qkv_pool.tile([128, KS, F], BF16, tag="v", name=f"v_{gc}_{cb}")
        for srct, dst in ((q, qsb), (k, ksb), (v, vsb)):
            nc.sync.dma_start(
                out=dst,
                in_=srct[c0 : c0 + 128, :]
                .rearrange("p (i g) -> p i g", i=KS)[:, :, g0 : g0 + F],
            )
        return {"qsb": qsb, "ksb": ksb, "vsb": vsb}

    def tmp_stage(blk, st):
        gc, cb = blk
        # tmp[i,j] = q_i * k_j  (DVE, one op, 2x)
        tmp = tmp_pool.tile([128, KS, KS, F], BF16, tag="tmp", name=f"tmp_{gc}_{cb}")
        nc.vector.tensor_mul(
            out=tmp,
            in0=st["qsb"].unsqueeze(2).broadcast_to([128, KS, KS, F]),
            in1=st["ksb"].unsqueeze(1).broadcast_to([128, KS, KS, F]),
        )
        st["tmp"] = tmp

    def scores_stage(blk, st):
        gc, cb = blk
        tmp = st["tmp"]
        # scores: j-outer so each W_j loads once; rows 32i+4j+h
        s_ps = ps_s.tile([96, F], F32, tag="S", name=f"S_{gc}_{cb}")
        for j in range(KS):
            for i in range(KS):
                nc.tensor.matmul(
                    s_ps[32 * i : 32 * i + 32, :],
                    lhsT=wsc_sb[:, j, :],
                    rhs=tmp[:, i, j, :],
                    start=(j == 0),
                    stop=(j == KS - 1),
                    skip_group_check=True,
                )
        st["s_ps"] = s_ps

    def exp_stage(blk, st):
        gc, cb = blk
        e_sb = sm_pool.tile([96, F], BF16, tag="E", name=f"E_{gc}_{cb}")
        nc.scalar.activation(
            out=e_sb,
            in_=st["s_ps"],
            func=mybir.ActivationFunctionType.Exp,
            scale=SCALE,
        )
        st["e_sb"] = e_sb

    def dmm_stage(blk, st):
        gc, cb = blk
        d_ps = ps_d.tile([96, F], F32, tag="D", name=f"D_{gc}_{cb}")
        nc.tensor.matmul(d_ps, lhsT=wd_sb, rhs=st["e_sb"], start=True, stop=True)
        st["d_ps"] = d_ps

    def recip_stage(blk, st):
        gc, cb = blk
        dinv = sm_pool.tile([96, F], F32, tag="Di", name=f"Di_{gc}_{cb}")
        nc.vector.reciprocal_approx_fast(out=dinv, in_=st["d_ps"])
        st["dinv"] = dinv

    def pmul_stage(blk, st):
        gc, cb = blk
        p_sb = sm_pool.tile([96, F], BF16, tag="P", name=f"P_{gc}_{cb}")
        nc.gpsimd.tensor_mul(out=p_sb, in0=st["e_sb"], in1=st["dinv"])
        st["p_sb"] = p_sb

    def dv_stage(blk, st):
        gc, cb = blk
        vsb = st["vsb"]
        dv = t4_pool.tile([128, 2, F], BF16, tag="dv", name=f"dv_{gc}_{cb}")
        nc.gpsimd.tensor_sub(
            out=dv,
            in0=vsb[:, 0:KS:2, :],
            in1=vsb[:, 1:2, :].broadcast_to([128, 2, F]),
        )
        st["dv"] = dv

    def br_mm(blk, st, r):
        gc, cb = blk
        br_ps = ps_br.tile([128, KS, F], F32, tag="Br", name=f"Br_{gc}_{cb}_{r}")
        for i in range(KS):
            nc.tensor.matmul(
                br_ps[:, i, :],
                lhsT=wbr_sb[:, 2 * i + r, :],
                rhs=st["p_sb"],
                start=True,
                stop=True,
            )
        st[f"br{r}"] = br_ps

    def br_evac(blk, st, r):
        gc, cb = blk
        br_sb = t4_pool.tile(
            [128, KS, F], BF16, tag=f"brsb{r}", name=f"brsb_{gc}_{cb}_{r}"
        )
        nc.scalar.copy(out=br_sb, in_=st[f"br{r}"])
        st[f"brsb{r}"] = br_sb

    def t4_mul(blk, st, r):
        gc, cb = blk
        t4_r = t4_pool.tile(
            [128, KS, F], BF16, tag=f"t4{r}", name=f"t4_{gc}_{cb}_{r}"
        )
        nc.vector.tensor_mul(
            out=t4_r,
            in0=st[f"brsb{r}"],
            in1=st["dv"][:, r : r + 1, :].broadcast_to([128, KS, F]),
        )
        st[f"t4{r}"] = t4_r

    def trans_stage(blk, st):
        gc, cb = blk
        vsb = st["vsb"]
        # transpose + accumulate: t_ps[g, (i, gs, c)] = (t4_i0+t4_i2+v1)^T
        t_ps = ps_t.tile([128, KS, F], F32, tag="T", name=f"T_{gc}_{cb}")
        for i in range(KS):
            for gs in range(GS):
                sl = slice(gs * 128, (gs + 1) * 128)
                for step, lhs in enumerate(
                    (st["t40"][:, i, sl], st["t41"][:, i, sl], vsb[:, 1, sl])
                ):
                    nc.tensor.matmul(
                        t_ps[:, i, sl],
                        lhsT=lhs,
                        rhs=id_sb,
                        start=(step == 0),
                        stop=(step == 2),
                    )
        st["t_ps"] = t_ps

    def uevac_stage(blk, st, osb):
        gc, cb = blk
        # out ch 384*(cb%2) + 96*hl + 32*i + cc, token t=cb//2
        t = cb // 2
        dst = (
            osb[:, :, t, 384 * (cb % 2) : 384 * (cb % 2) + 384]
            .rearrange("p gs (hl i cc) -> p i gs hl cc", hl=4, i=KS)
        )
        src = st["t_ps"].rearrange("p i (gs hl cc) -> p i gs hl cc", gs=GS, hl=4)
        # ISA allows at most 3 free dims per AP (and gpsimd cannot read
        # PSUM): one ACT copy per i
        for i in range(KS):
            nc.scalar.copy(out=dst[:, i], in_=src[:, i])

    # 7-deep software pipeline. Stage of block m runs at iteration m + off:
    #   loads +0 | tmp +1 | scores,exp +2 | D,recip,Pmul +3 |
    #   dv,br,evac,t4 +4 | trans +5 | uevac +6
    # Emission order per iteration puts ready-at-emission work first in
    # each engine queue.
    blocks = [(gc, cb) for gc in range(GC) for cb in range(CB)]
    osb_by_gc = {
        gc: out_pool.tile([128, GS, KS, D], FP16, tag="osb", name=f"osb_{gc}")
        for gc in range(GC)
    }
    state: dict = {}
    nb = len(blocks)

    def blk_at(off, n):
        m = n - off
        return blocks[m] if 0 <= m < nb else None

    for n in range(nb + 7):
        if (b := blk_at(6, n)) is not None:  # ACT: ready (trans last iter)
            uevac_stage(b, state[b], osb_by_gc[b[0]])
            if b[1] == CB - 1:
                nc.scalar.dma_start(out=out_r[b[0]], in_=osb_by_gc[b[0]])
            state.pop(b)
        if (b := blk_at(4, n)) is not None:  # PE: ready (Pmul last iter)
            br_mm(b, state[b], 0)
        if (b := blk_at(4, n)) is not None:  # ACT: after brA this iter
            br_evac(b, state[b], 0)
        if (b := blk_at(2, n)) is not None:  # PE: ready (tmp last iter)
            scores_stage(b, state[b])
        if (b := blk_at(4, n)) is not None:  # PE: br tile free after evacA
            br_mm(b, state[b], 1)
        if (b := blk_at(2, n)) is not None:  # ACT: after scores this iter
            exp_stage(b, state[b])
        if (b := blk_at(4, n)) is not None:  # ACT: after brB this iter
            br_evac(b, state[b], 1)
        if (b := blk_at(3, n)) is not None:  # PE: ready (exp last iter)
            dmm_stage(b, state[b])
        if (b := blk_at(5, n)) is not None:  # PE: ready (t4 last iter)
            trans_stage(b, state[b])
        if (b := blk_at(1, n)) is not None:  # DVE: ready (loads last iter)
            tmp_stage(b, state[b])
        if (b := blk_at(4, n)) is not None:  # DVE: ready (v loaded)
            dv_stage(b, state[b])
            t4_mul(b, state[b], 0)  # after evacA this iter
        if (b := blk_at(3, n)) is not None:  # DVE: after D this iter
            recip_stage(b, state[b])
        if (b := blk_at(4, n)) is not None:  # DVE: after evacB this iter
            t4_mul(b, state[b], 1)
        if (b := blk_at(3, n)) is not None:  # Pool: after recip this iter
            pmul_stage(b, state[b])
        if (b := blk_at(0, n)) is not None:  # SP
            state[b] = loads(b)


def _get_nc():
    if "nc" in _CACHE:
        return _CACHE["nc"]
    nc = bacc.Bacc("TRN2", target_bir_lowering=False, debug=False, num_devices=NCORES)
    q = nc.dram_tensor("q", [D, N], BF16, kind="ExternalInput").ap()
    k = nc.dram_tensor("k", [D, N], BF16, kind="ExternalInput").ap()
    v = nc.dram_tensor("v", [D, N], BF16, kind="ExternalInput").ap()
    out = nc.dram_tensor("out", [N, D], FP16, kind="ExternalOutput").ap()
    wsc = nc.dram_tensor("wsc", [KS, 128, 32], BF16, kind="ExternalInput").ap()
    wd = nc.dram_tensor("wd", [96, 96], BF16, kind="ExternalInput").ap()
    wbr = nc.dram_tensor("wbr", [len(IJ), 96, 128], BF16, kind="ExternalInput").ap()
    ident = nc.dram_tensor("ident", [128, 128], BF16, kind="ExternalInput").ap()
    with tile.TileContext(nc) as tc:
        with ExitStack() as ctx:
            _build_kernel(ctx, tc, q, k, v, out, wsc, wd, wbr, ident)
    nc.compile()
    _CACHE["nc"] = nc
    return nc


def kernel(q, k, v, head_dim, kernel_size, _trace=False, _trace_kwargs=None):
    assert int(head_dim) == HD and int(kernel_size) == KS
    bf = mybir.dt.np(BF16)
    q = np.asarray(q, dtype=np.float32).astype(bf)
    k = np.asarray(k, dtype=np.float32).astype(bf)
    v = np.asarray(v, dtype=np.float32).astype(bf)
    assert q.shape == (B, D, N)

    nc = _get_nc()
    wsc, wd, wbr, ident = _host_consts()
    consts = {
        "wsc": wsc.astype(bf),
        "wd": wd.astype(bf),
        "wbr": wbr.astype(bf),
        "ident": ident.astype(bf),
    }
    in_maps = [{"q": q[b], "k": k[b], "v": v[b], **consts} for b in range(B)]
    res = run_bass_kernel_spmd(
        nc,
        in_maps,
        core_ids=list(range(NCORES)),
        trace=_trace,
        **(_trace_kwargs or {}),
    )
    out = np.stack(
        [res.results[b]["out"].astype(np.float32) for b in range(B)], axis=0
    )
    _CACHE["last_results"] = res
    return out


if __name__ == "__main__":
    rng = np.random.default_rng(0)
    qq = rng.standard_normal((B, D, N), dtype=np.float32)
    kk = rng.standard_normal((B, D, N), dtype=np.float32)
    vv = rng.standard_normal((B, D, N), dtype=np.float32)
    o = kernel(qq, kk, vv, HD, KS)
    print("out", o.shape, o.dtype, float(np.abs(o).max()))
